# revision 19
# baseline (speedup 1.0000x reference)
"""Trainium2 Bass kernel for nn_CLFMv2_NoTemporalEmb (graph-PDE message passing).

Strategy: data-parallel over batch B=64 across 8 NeuronCores (8 batches/core).
Per core, activations are "pair-packed feature-major":
    tensor[psi, n],  psi = (batch_parity)*64 + d  (128 partitions),
    one [128, 1024] tensor per batch-pair (4 pairs/core).
Weight matmuls use block-diagonal [128,128] stationary operands so K=128,
M=128. The Laplacian GEMM runs in fp8-e4m3 DoubleRow mode (K=256 per pass,
2x bf16 rate): the host packs S_M*(gamma/kappa)*A^T (diagonal-free,
row-stochastic A so quantization noise averages over K=1024), and the
PE-transposed field tiles are quantized to fp8 (x S_F) on the fly by the
PSUM->SBUF copy.  All "+field"/"+bias" affine plumbing is folded host-side:
the carried tensor is f~ = kappa*field minus a per-step offset vector, so
the fe-combine and field-update are single DVE scalar_tensor_tensor ops
(out = in0*scalar + in1) and no identity-matmul adds are needed.
Matmuls otherwise run in bf16; PSUM accumulates fp32.
"""

import contextlib

import numpy as np

import concourse.bacc as bacc
import concourse.tile as tile
import concourse.mybir as mybir
from concourse.bass_utils import run_bass_kernel_spmd

F32 = mybir.dt.float32
BF16 = mybir.dt.bfloat16
FP8 = mybir.dt.float8e4
MMDT = BF16
AF = mybir.ActivationFunctionType
ALU = mybir.AluOpType
DR = mybir.MatmulPerfMode.DoubleRow

B, L, N, D, H, O = 64, 12, 1024, 64, 128, 12
STEPS = 4
NCORES = 8
BL = B // NCORES          # 8 batches per core
PAIRS = BL // 2           # 4
KCH = N // 128            # 8 adjacency chunks
KPAIR = KCH // 2          # 4 DoubleRow chunk-pairs
SF = 4.0                  # fp8 scale on transposed-field tiles
SM = 512.0                # fp8 scale on the adjacency operator
SINV = 1.0 / (SF * SM)

# weight-pack slot order (each slot is a [128, 128] block in wpk)
WNAMES = ["w1eA", "w1eB", "w2eA", "w2eB", "pw1A", "pw1B",
          "wzbd", "uzbd", "whbd", "uhbd", "wobd", "dw1A", "dw1B",
          "dw2A", "dw2B", "ieye"]
BNAMES = (["eb1A", "eb1B"]
          + [f"pb1A_{s}" for s in range(STEPS)]
          + [f"pb1B_{s}" for s in range(STEPS)]
          + [f"bz_{s}" for s in range(STEPS)]
          + [f"bh_{s}" for s in range(STEPS)]
          + ["db1A", "db1B", "db2"])


def _build(kappa):
    nc = bacc.Bacc("TRN2", target_bir_lowering=False, debug=False)

    wpk = nc.dram_tensor("wpk", [128, len(WNAMES) * 128], MMDT,
                         kind="ExternalInput")
    bpk = nc.dram_tensor("bpk", [128, len(BNAMES)], F32, kind="ExternalInput")
    hist = nc.dram_tensor("hist", [BL, L, N], MMDT, kind="ExternalInput")
    ath = nc.dram_tensor("ath", [128, KPAIR, 2, N], FP8, kind="ExternalInput")
    pw2 = nc.dram_tensor("pw2", [128, 2, 128], FP8, kind="ExternalInput")
    out = nc.dram_tensor("out", [BL, O, N], F32, kind="ExternalOutput")

    with tile.TileContext(nc) as tc:
        with contextlib.ExitStack() as ctx:
            pp = ctx.enter_context(tc.tile_pool(name="persist", bufs=1))
            hab = ctx.enter_context(tc.tile_pool(name="hab", bufs=10))
            ftp = ctx.enter_context(tc.tile_pool(name="ftp", bufs=5))
            fep = ctx.enter_context(tc.tile_pool(name="fep", bufs=7))
            zcp = ctx.enter_context(tc.tile_pool(name="zcp", bufs=4))
            tmp = ctx.enter_context(tc.tile_pool(name="tmp", bufs=2))
            x2p = ctx.enter_context(tc.tile_pool(name="x2p", bufs=2))
            o2p = ctx.enter_context(tc.tile_pool(name="o2p", bufs=2))
            psA = ctx.enter_context(tc.tile_pool(name="psA", bufs=2, space="PSUM"))
            psB = ctx.enter_context(tc.tile_pool(name="psB", bufs=2, space="PSUM"))

            # ---- PE warmup: ~3.4us of matmul activity flips the HAM
            # clock gate to 8/8 before the encoder's real matmuls arrive ----
            wsc = tmp.tile([128, N], MMDT, tag="tmp", name="wsc")
            nc.vector.memset(wsc[0:128, 0:512], 0.0)
            pwu = psB.tile([128, N], F32, tag="psB", name="pwu")
            for _ in range(8):
                nc.tensor.matmul(pwu[:, 0:512], wsc[:, 0:128], wsc[:, 0:512],
                                 start=True, stop=True)

            # ---- packed weights and biases: two DMAs ----
            wpkt = pp.tile([128, len(WNAMES) * 128], MMDT, tag="wpk", name="wpkt")
            nc.sync.dma_start(wpkt[:, 0:512], wpk[:, 0:512])
            nc.sync.dma_start(wpkt[:, 512:], wpk[:, 512:])
            bpkt = pp.tile([128, len(BNAMES)], F32, tag="bpk", name="bpkt")
            nc.sync.dma_start(bpkt[:], bpk[:, :])
            pw2t = pp.tile([128, 2, 128], FP8, tag="pw2", name="pw2t")
            nc.sync.dma_start(pw2t[:], pw2[:, :, :])

            wt = {}
            for i, name in enumerate(WNAMES):
                if name in ("w1eA", "w1eB"):
                    wt[name] = wpkt[0:2 * L, i * 128:(i + 1) * 128]
                elif name in ("dw2A", "dw2B"):
                    wt[name] = wpkt[:, i * 128:i * 128 + 2 * O]
                else:
                    wt[name] = wpkt[:, i * 128:(i + 1) * 128]
            bs = {}
            for j, name in enumerate(BNAMES):
                if name == "db2":
                    bs[name] = bpkt[0:2 * O, j:j + 1]
                else:
                    bs[name] = bpkt[:, j:j + 1]

            # per-pair persistent activations (f~ and state)
            field = [pp.tile([128, N], MMDT, tag=f"field{p}", name=f"field{p}")
                     for p in range(PAIRS)]
            state = [pp.tile([128, N], MMDT, tag=f"state{p}", name=f"state{p}")
                     for p in range(PAIRS)]

            # ---- encoder (emitted before the big AT DMA) ----
            for p in range(PAIRS):
                xp = x2p.tile([2 * L, N], MMDT, tag="x2p", name="xp")
                nc.sync.dma_start(xp[0:L, :], hist[2 * p, :, :])
                nc.sync.dma_start(xp[L:2 * L, :], hist[2 * p + 1, :, :])
                hea = hab.tile([128, N], MMDT, tag="hab", name="hea")
                heb = hab.tile([128, N], MMDT, tag="hab", name="heb")
                for (wname, bname, dst) in [("w1eA", "eb1A", hea),
                                            ("w1eB", "eb1B", heb)]:
                    ph = psA.tile([128, N], F32, tag="psA", name="psah")
                    for hf in range(2):
                        sl = slice(hf * 512, (hf + 1) * 512)
                        nc.tensor.matmul(ph[:, sl], wt[wname], xp[:, sl],
                                         start=True, stop=True)
                    nc.scalar.activation(dst[:], ph[:], AF.Relu, bias=bs[bname])
                pf = psB.tile([128, N], F32, tag="psB", name="psbf")
                for hf in range(2):
                    sl = slice(hf * 512, (hf + 1) * 512)
                    nc.tensor.matmul(pf[:, sl], wt["w2eA"], hea[:, sl],
                                     start=True, stop=False)
                    nc.tensor.matmul(pf[:, sl], wt["w2eB"], heb[:, sl],
                                     start=False, stop=True)
                # f~_nb = kappa*(field - enc_b2): enc_w2 pre-scaled, no bias
                nc.scalar.activation(field[p][:], pf[:], AF.Copy)

            # ---- adjacency operator: host-precomputed fp8, one DMA ----
            AT = pp.tile([128, KPAIR, 2, N], FP8, tag="AT", name="AT")
            nc.sync.dma_start(AT[:], ath[:, :, :, :])

            def emit_transpose(p):
                ptr = psA.tile([128, N], F32, tag="psA", name="psatr")
                # fp8-quantized (x SF) transposed field, [m_local, chunk, psi];
                # half-granular so the DoubleRow matmuls (which only need
                # chunks 2kp..2kp+1 each) can start after half0.
                ft = ftp.tile([128, KCH, 128], FP8, tag="ft", name="ft")
                for hf in range(2):
                    for k in range(4 * hf, 4 * hf + 4):
                        nc.tensor.matmul(ptr[:, k * 128:(k + 1) * 128],
                                         field[p][:, k * 128:(k + 1) * 128],
                                         wt["ieye"], start=True, stop=True)
                    sl = slice(hf * 512, (hf + 1) * 512)
                    nc.vector.tensor_scalar(ft[:, 4 * hf:4 * hf + 4, :],
                                            ptr[:, sl], SF, None, ALU.mult)
                return ft

            ftq = [emit_transpose(p) for p in range(PAIRS)]

            # ---- main steps: per-pair wavefront across step boundaries ----
            def emit_pde1(s, p):
                # pde layer 1: h = tanh(f~ @ pw1_eff + pb1_eff_s), written as
                # fp8 interleaved [128, {A,B}, N] for the DoubleRow pde2 matmul
                hq = hab.tile([128, 2, N], FP8, tag="hab", name="hq")
                for i, (wname, bname) in enumerate([("pw1A", f"pb1A_{s}"),
                                                    ("pw1B", f"pb1B_{s}")]):
                    ph = psA.tile([128, N], F32, tag="psA", name="psah")
                    for hf in range(2):
                        sl = slice(hf * 512, (hf + 1) * 512)
                        nc.tensor.matmul(ph[:, sl], wt[wname],
                                         field[p][:, sl],
                                         start=True, stop=True)
                    nc.scalar.activation(hq[:, i, :], ph[:], AF.Tanh,
                                         bias=bs[bname])
                return hq

            def emit_pfe(p, ft, hq):
                # fe psum: fp8 DoubleRow Laplacian first (no tanh dependency),
                # then DoubleRow pde layer 2; fe_nb = pfe/S + f~_nb on DVE.
                fe_t = fep.tile([128, N], MMDT, tag="fe", name="fe_t")
                pfe = psB.tile([128, N], F32, tag="psB", name="psbfe")
                for hf in range(2):
                    sl = slice(hf * 512, (hf + 1) * 512)
                    for kp in range(KPAIR):
                        nc.tensor.matmul(
                            pfe[:, sl],
                            ft[:, 2 * kp:2 * kp + 2, :],
                            AT[:, kp, :, sl],
                            start=(kp == 0), stop=False,
                            perf_mode=DR)
                for hf in range(2):
                    sl = slice(hf * 512, (hf + 1) * 512)
                    nc.tensor.matmul(pfe[:, sl], pw2t[:, :, :], hq[:, :, sl],
                                     start=False, stop=True, perf_mode=DR)
                    nc.vector.scalar_tensor_tensor(
                        fe_t[:, sl], pfe[:, sl], SINV, field[p][:, sl],
                        ALU.mult, ALU.add)
                return fe_t

            def emit_gru(s, p, fe_t):
                first = (s == 0)
                z_t = zcp.tile([128, N], MMDT, tag="zc", name="z_t")
                c_t = zcp.tile([128, N], MMDT, tag="zc", name="c_t")
                # c-gate first so the state chain (sub needs c) starts while
                # the z-gate matmuls/sigmoid still run; halved activations
                for (wname, uname, bname, func, dst) in [
                    ("whbd", "uhbd", f"bh_{s}", AF.Tanh, c_t),
                    ("wzbd", "uzbd", f"bz_{s}", AF.Sigmoid, z_t),
                ]:
                    pz = psB.tile([128, N], F32, tag="psB", name="psbz")
                    for hf in range(2):
                        sl = slice(hf * 512, (hf + 1) * 512)
                        nc.tensor.matmul(pz[:, sl], wt[wname], fe_t[:, sl],
                                         start=True, stop=first)
                        if not first:
                            nc.tensor.matmul(pz[:, sl], wt[uname],
                                             state[p][:, sl],
                                             start=False, stop=True)
                    nc.scalar.activation(dst[:], pz[:], func, bias=bs[bname])
                t1 = None if first else tmp.tile([128, N], MMDT, tag="tmp",
                                                 name="t1")
                for hf in range(2):
                    sl = slice(hf * 512, (hf + 1) * 512)
                    if first:
                        nc.vector.tensor_tensor(state[p][:, sl], z_t[:, sl],
                                                c_t[:, sl], ALU.mult)
                    else:
                        nc.vector.tensor_tensor(t1[:, sl], c_t[:, sl],
                                                state[p][:, sl], ALU.subtract)
                        nc.vector.tensor_tensor(t1[:, sl], z_t[:, sl],
                                                t1[:, sl], ALU.mult)
                        nc.vector.tensor_tensor(state[p][:, sl],
                                                state[p][:, sl], t1[:, sl],
                                                ALU.add)

            def emit_fieldupd(p, fe_t):
                # f~' = kappa*fe + state' @ (kappa*wo)  (psum + DVE halves)
                pf = psB.tile([128, N], F32, tag="psB", name="psbf2")
                for hf in range(2):
                    sl = slice(hf * 512, (hf + 1) * 512)
                    nc.tensor.matmul(pf[:, sl], wt["wobd"], state[p][:, sl],
                                     start=True, stop=True)
                    nc.vector.scalar_tensor_tensor(
                        field[p][:, sl], fe_t[:, sl], kappa, pf[:, sl],
                        ALU.mult, ALU.add)

            def emit_dec(p):
                dha = hab.tile([128, N], MMDT, tag="hab", name="dha")
                dhb = hab.tile([128, N], MMDT, tag="hab", name="dhb")
                for (wname, bname, dst) in [("dw1A", "db1A", dha),
                                            ("dw1B", "db1B", dhb)]:
                    ph = psA.tile([128, N], F32, tag="psA", name="psah")
                    for hf in range(2):
                        sl = slice(hf * 512, (hf + 1) * 512)
                        nc.tensor.matmul(ph[:, sl], wt[wname],
                                         field[p][:, sl],
                                         start=True, stop=True)
                    nc.scalar.activation(dst[:], ph[:], AF.Relu,
                                         bias=bs[bname])
                po = psB.tile([2 * O, N], F32, tag="psB", name="psbo")
                for hf in range(2):
                    sl = slice(hf * 512, (hf + 1) * 512)
                    nc.tensor.matmul(po[:, sl], wt["dw2A"], dha[:, sl],
                                     start=True, stop=False)
                    nc.tensor.matmul(po[:, sl], wt["dw2B"], dhb[:, sl],
                                     start=False, stop=True)
                o2 = o2p.tile([2 * O, N], F32, tag="o2", name="o2")
                nc.scalar.activation(o2[:], po[:], AF.Identity, bias=bs["db2"])
                nc.sync.dma_start(out[2 * p, :, :], o2[0:O, :])
                nc.sync.dma_start(out[2 * p + 1, :, :], o2[O:2 * O, :])

            # software pipeline: fts/has/fes keyed per (step, pair); each
            # pair's next-step front work (transpose + pde1 + pfe) is emitted
            # as soon as its own deps allow, so no engine FIFO holds a
            # stalled op in front of ready work.
            fts = {}
            has = {}
            fes = {}
            for p in range(PAIRS):
                fts[(0, p)] = ftq[p]
            for p in range(PAIRS):
                has[(0, p)] = emit_pde1(0, p)
            for p in range(PAIRS):
                fes[(0, p)] = emit_pfe(p, fts[(0, p)], has[(0, p)])

            for s in range(STEPS):
                last = (s == STEPS - 1)

                def emit_eb(p):
                    # after gru(p): field update, then next-step transpose +
                    # pde1 (or the decoder on the last step)
                    emit_fieldupd(p, fes[(s, p)])
                    if not last:
                        fts[(s + 1, p)] = emit_transpose(p)
                        has[(s + 1, p)] = emit_pde1(s + 1, p)
                    else:
                        emit_dec(p)

                def emit_front(p):
                    if not last:
                        fes[(s + 1, p)] = emit_pfe(p, fts[(s + 1, p)],
                                                   has[(s + 1, p)])

                emit_gru(s, 0, fes[(s, 0)])
                emit_gru(s, 1, fes[(s, 1)])
                emit_eb(0)
                emit_gru(s, 2, fes[(s, 2)])
                emit_front(0)
                emit_eb(1)
                emit_gru(s, 3, fes[(s, 3)])
                emit_front(1)
                emit_eb(2)
                emit_front(2)
                emit_eb(3)
                emit_front(3)

    nc.compile()
    return nc


MMNP = mybir.dt.np(MMDT)
FP8NP = mybir.dt.np(FP8)


def _blockdiag(w):
    w = np.asarray(w, dtype=np.float64)
    r, c = w.shape
    o = np.zeros((2 * r, 2 * c), dtype=np.float64)
    o[:r, :c] = w
    o[r:, c:] = w
    return o


def _slot(w):
    """place an array into a [128, 128] weight slot."""
    w = np.asarray(w, dtype=np.float64)
    o = np.zeros((128, 128), dtype=np.float64)
    o[:w.shape[0], :w.shape[1]] = w
    return o


def prepare(inputs):
    """Host packing (float64) + compiled Bass module + per-core input maps."""
    g = {k: np.asarray(v) for k, v in inputs.items()}
    pde_mix = float(np.asarray(g["pde_mix"], dtype=np.float64))
    alpha = float(1.0 / (1.0 + np.exp(-pde_mix)))
    dt_ = 1.0 / STEPS
    s2 = (1.0 - alpha) * dt_
    gam = alpha * dt_
    kap = 1.0 - gam

    f64 = lambda k: np.asarray(g[k], np.float64)
    enc_w1, enc_w2 = f64("enc_w1"), f64("enc_w2") * kap
    pde_w1, pde_w2 = f64("pde_w1") / kap, f64("pde_w2") * (s2 * SF * SM)
    pw2q = np.stack([_slot(_blockdiag(pde_w2[0:64, :])),
                     _slot(_blockdiag(pde_w2[64:128, :]))], axis=1)
    dec_w1, dec_w2 = f64("dec_w1") / kap, f64("dec_w2")

    slots = {
        "w1eA": _blockdiag(enc_w1[:, 0:64]),
        "w1eB": _blockdiag(enc_w1[:, 64:128]),
        "w2eA": _blockdiag(enc_w2[0:64, :]),
        "w2eB": _blockdiag(enc_w2[64:128, :]),
        "pw1A": _blockdiag(pde_w1[:, 0:64]),
        "pw1B": _blockdiag(pde_w1[:, 64:128]),
        "wzbd": _blockdiag(f64("ss_wz")),
        "uzbd": _blockdiag(f64("ss_uz")),
        "whbd": _blockdiag(f64("ss_wh")),
        "uhbd": _blockdiag(f64("ss_uh")),
        "wobd": _blockdiag(f64("ss_wo") * kap),
        "dw1A": _blockdiag(dec_w1[:, 0:64]),
        "dw1B": _blockdiag(dec_w1[:, 64:128]),
        "dw2A": _blockdiag(dec_w2[0:64, :]),
        "dw2B": _blockdiag(dec_w2[64:128, :]),
        "ieye": np.eye(128, dtype=np.float64),
    }
    wpk = np.concatenate([_slot(slots[n]) for n in WNAMES], axis=1)

    # per-step bias folding: carried tensor is f~_nb = kap*field - v~ (v~ per-d
    # offset vector); fe_nb = fe - u with u = v~*(1+gam/kap) + s2*pde_b2.
    bias_vals = {
        "eb1A": np.tile(f64("enc_b1")[0:64], 2),
        "eb1B": np.tile(f64("enc_b1")[64:128], 2),
    }
    vt = kap * f64("enc_b2")
    for s in range(STEPS):
        pb1 = f64("pde_b1") + (vt / kap) @ f64("pde_w1")
        bias_vals[f"pb1A_{s}"] = np.tile(pb1[0:64], 2)
        bias_vals[f"pb1B_{s}"] = np.tile(pb1[64:128], 2)
        u = vt * (1.0 + gam / kap) + s2 * f64("pde_b2")
        bias_vals[f"bz_{s}"] = np.tile(f64("ss_bz") + u @ f64("ss_wz"), 2)
        bias_vals[f"bh_{s}"] = np.tile(f64("ss_bh") + u @ f64("ss_wh"), 2)
        vt = kap * u + kap * f64("ss_bo")
    db1 = f64("dec_b1") + (vt / kap) @ f64("dec_w1")
    bias_vals["db1A"] = np.tile(db1[0:64], 2)
    bias_vals["db1B"] = np.tile(db1[64:128], 2)
    bias_vals["db2"] = np.tile(f64("dec_b2"), 2)

    bpk = np.zeros((128, len(BNAMES)), dtype=np.float64)
    for j, name in enumerate(BNAMES):
        v = bias_vals[name]
        bpk[:len(v), j] = v

    # adjacency operator: softmax rows, scale, transpose, fp8 DoubleRow pack
    adj64 = f64("adj")
    e = np.exp(adj64 - adj64.max(axis=-1, keepdims=True))
    A = e / e.sum(axis=-1, keepdims=True)
    M = (SM * gam / kap) * A
    # ath[p, kp, i, n] = M[n, (2*kp + i)*128 + p]
    ath = M.T.reshape(KPAIR, 2, 128, N).transpose(2, 0, 1, 3)

    common = {
        "wpk": np.ascontiguousarray(wpk.astype(np.float32)).astype(MMNP),
        "bpk": np.ascontiguousarray(bpk.astype(np.float32)),
        "ath": np.ascontiguousarray(ath.astype(np.float32)).astype(FP8NP),
        "pw2": np.ascontiguousarray(np.clip(pw2q, -240, 240)
                                    .astype(np.float32)).astype(FP8NP),
    }

    hist = np.asarray(g["history_data"], np.float32)[..., 0]  # [B, L, N]
    in_maps = []
    for c in range(NCORES):
        m = dict(common)
        m["hist"] = np.ascontiguousarray(hist[c * BL:(c + 1) * BL]).astype(MMNP)
        in_maps.append(m)

    nc = _build(kap)
    return nc, in_maps


def assemble(results):
    outs = [results[c]["out"] for c in range(NCORES)]          # [BL, O, N]
    full = np.concatenate(outs, axis=0)                        # [B, O, N]
    return np.ascontiguousarray(full[..., None].astype(np.float32))


def kernel(**inputs) -> np.ndarray:
    nc, in_maps = prepare(inputs)
    res = run_bass_kernel_spmd(nc, in_maps, core_ids=list(range(NCORES)))
    return assemble(res.results)
